# revision 1
# baseline (speedup 1.0000x reference)
"""Trainium2 Bass kernel for edge-softmax attention aggregation (GNN message passing).

Strategy: destination-sharded segment softmax.
  - Host: assign each destination node to a (core, block, col-strip, slot)
    position; permute edges so each 32-node "subblock" owns a contiguous,
    128*C_SUB-padded run of edges.  cutoff/sqrt(dk) is folded into q.
  - Device (per core, SPMD over 8 cores): stream q|k|v packed rows, compute
    per-edge per-head logits w = sum_j q_j k_j on DVE, exp on ACT, s*v on DVE,
    build a [128e x 32n] one-hot from slot ids with one tensor_scalar(is_equal),
    and scatter-add [exp | exp*v] into PSUM with TensorE matmuls (col-tiled,
    4 strips of 32 nodes per 128-node block).  Per block: out = num / den.
  - Host: inverse-permute rows to original node order.
"""

import sys

if "/opt/trn_rl_repo" not in sys.path:
    sys.path.insert(0, "/opt/trn_rl_repo")

import numpy as np

import concourse.bacc as bacc
import concourse.mybir as mybir
import concourse.tile as tile
from concourse.bass_utils import run_bass_kernel_spmd

F32 = mybir.dt.float32

N_NODES = 50000
N_EDGES = 1_600_000
DK = 64
H = 8
DH = 8  # per-head dim
NC = 8  # cores

SUB_NODES = 32      # node slots per subblock (= one-hot width = matmul M)
SUBS_PER_BLOCK = 4  # PSUM col strips per 128-node block
DEFAULT_BLOCKS = 50  # 128-node blocks per core


def build_program(c_sub: int, blocks: int, n_cores: int):
    """Build + compile the SPMD Bass program (one program, all cores)."""
    g_core = blocks * SUBS_PER_BLOCK        # subblock groups per core
    slots_sub = 128 * c_sub                 # edge slots per subblock

    nc = bacc.Bacc("TRN2", target_bir_lowering=False, debug=False,
                   num_devices=n_cores)
    qkv = nc.declare_dram_parameter(
        "qkv", [g_core * slots_sub, 192], F32, isOutput=False)
    lidx = nc.declare_dram_parameter(
        "lidx", [128, g_core * c_sub], F32, isOutput=False)
    iota = nc.declare_dram_parameter("iota", [128, SUB_NODES], F32,
                                     isOutput=False)
    out = nc.declare_dram_parameter("out", [blocks * 128, DK], F32,
                                    isOutput=True)

    with tile.TileContext(nc) as tc:
        with (
            tc.tile_pool(name="const", bufs=1) as cpool,
            tc.tile_pool(name="io", bufs=3) as iopool,
            tc.tile_pool(name="work", bufs=3) as wpool,
            tc.tile_pool(name="psum", bufs=2, space="PSUM") as ppool,
            tc.tile_pool(name="outp", bufs=3) as opool,
        ):
            iota_t = cpool.tile([128, SUB_NODES], F32)
            nc.sync.dma_start(iota_t[:], iota[:])
            lidx_t = cpool.tile([128, g_core * c_sub], F32)
            nc.sync.dma_start(lidx_t[:], lidx[:])

            psum_t = None
            for g in range(g_core):
                j = g % SUBS_PER_BLOCK
                if j == 0:
                    psum_t = ppool.tile([128, 8 + DK], F32)

                dt = iopool.tile([128, c_sub, 192], F32)
                nc.sync.dma_start(
                    dt[:],
                    qkv[g * slots_sub:(g + 1) * slots_sub, :]
                    .rearrange("(p s) d -> p s d", p=128),
                )

                # per-edge, per-head logits
                qk = wpool.tile([128, c_sub, DK], F32)
                nc.vector.tensor_tensor(
                    qk[:], dt[:, :, 0:64], dt[:, :, 64:128],
                    op=mybir.AluOpType.mult)
                w = wpool.tile([128, c_sub, H], F32)
                nc.vector.tensor_reduce(
                    w[:], qk[:].rearrange("p s (h d) -> p s h d", d=DH),
                    axis=mybir.AxisListType.X, op=mybir.AluOpType.add)

                # rhs = [exp(w) | exp(w)*v] : [128, c_sub, 72]
                rhs = wpool.tile([128, c_sub, 8 + DK], F32)
                nc.scalar.activation(rhs[:, :, 0:8], w[:],
                                     mybir.ActivationFunctionType.Exp)
                nc.vector.tensor_tensor(
                    rhs[:, :, 8:72].rearrange("p s (h d) -> p s h d", d=DH),
                    dt[:, :, 128:192].rearrange("p s (h d) -> p s h d", d=DH),
                    rhs[:, :, 0:8].rearrange("p s (h o) -> p s h o", o=1)
                    .to_broadcast([128, c_sub, H, DH]),
                    op=mybir.AluOpType.mult)

                # scatter-add into the block accumulator via one-hot matmuls
                oh = wpool.tile([128, c_sub, SUB_NODES], F32)
                for s in range(c_sub):
                    col = g * c_sub + s
                    nc.vector.tensor_scalar(
                        out=oh[:, s, :], in0=iota_t[:],
                        scalar1=lidx_t[:, col:col + 1], scalar2=None,
                        op0=mybir.AluOpType.is_equal)
                    nc.tensor.matmul(
                        psum_t[32 * j:32 * (j + 1), :],
                        lhsT=oh[:, s, :], rhs=rhs[:, s, :],
                        start=(s == 0), stop=(s == c_sub - 1),
                        tile_position=(0, 32 * j))

                if j == SUBS_PER_BLOCK - 1:
                    b = g // SUBS_PER_BLOCK
                    rden = wpool.tile([128, H], F32)
                    nc.vector.reciprocal(rden[:], psum_t[:, 0:8])
                    ot = opool.tile([128, H, DH], F32)
                    nc.vector.tensor_tensor(
                        ot[:],
                        psum_t[:, 8:72].rearrange("p (h d) -> p h d", d=DH),
                        rden[:].rearrange("p (h o) -> p h o", o=1)
                        .to_broadcast([128, H, DH]),
                        op=mybir.AluOpType.mult)
                    nc.sync.dma_start(
                        out[b * 128:(b + 1) * 128, :],
                        ot[:].rearrange("p h d -> p (h d)"))

    nc.compile()
    return nc


def prepare(key, value, query, edge_weight_cutoff, edge_index,
            blocks=DEFAULT_BLOCKS, n_cores=NC):
    """Host-side sharding: node->slot assignment, edge permutation, packing."""
    n_nodes = N_NODES
    n_edges = edge_index.shape[1]
    nsb = n_cores * blocks * SUBS_PER_BLOCK  # total subblocks

    dst = np.asarray(edge_index[1], dtype=np.int64)
    deg = np.bincount(dst, minlength=n_nodes)

    # snake-deal nodes (sorted by degree desc) into nsb bins -> balanced edges
    order_nodes = np.argsort(-deg, kind="stable")
    rounds = -(-n_nodes // nsb)
    assert rounds <= SUB_NODES, "too few subblocks for node count"
    padded = np.full(rounds * nsb, -1, dtype=np.int64)
    padded[:n_nodes] = order_nodes
    arr = padded.reshape(rounds, nsb)
    arr[1::2] = arr[1::2, ::-1]  # snake
    bin_of_node = np.empty(n_nodes, dtype=np.int64)
    slot_of_node = np.empty(n_nodes, dtype=np.int64)
    rr, cc = np.divmod(np.arange(rounds * nsb), nsb)
    flat = arr.reshape(-1)
    mask = flat >= 0
    bin_of_node[flat[mask]] = cc[mask]
    slot_of_node[flat[mask]] = rr[mask]

    bin_edges = np.bincount(bin_of_node[dst], minlength=nsb)
    c_sub = max(1, int(-(-bin_edges.max() // 128)))
    slots_sub = 128 * c_sub

    # group edges by subblock, pad each subblock to slots_sub
    sb_of_edge = bin_of_node[dst]
    eorder = np.argsort(sb_of_edge, kind="stable")
    counts = np.bincount(sb_of_edge, minlength=nsb)
    offsets = np.zeros(nsb + 1, dtype=np.int64)
    np.cumsum(counts, out=offsets[1:])
    sb_sorted = sb_of_edge[eorder]
    rank = np.arange(n_edges, dtype=np.int64) - offsets[sb_sorted]
    pos = sb_sorted * slots_sub + rank

    perm = np.full(nsb * slots_sub, n_edges, dtype=np.int64)
    perm[pos] = eorder
    lidx_flat = np.full(nsb * slots_sub, float(SUB_NODES + 7), dtype=np.float32)
    lidx_flat[pos] = slot_of_node[dst[eorder]].astype(np.float32)

    # pack q*cutoff/sqrt(dh) | k | v, with a zero row for padding slots
    scale = (np.asarray(edge_weight_cutoff, np.float32)
             * np.float32(1.0 / np.sqrt(DH)))
    packed = np.empty((n_edges + 1, 192), dtype=np.float32)
    packed[:n_edges, 0:64] = np.asarray(query, np.float32) * scale[:, None]
    packed[:n_edges, 64:128] = np.asarray(key, np.float32)
    packed[:n_edges, 128:192] = np.asarray(value, np.float32)
    packed[n_edges] = 0.0

    qkv_dev = packed[perm]  # [nsb*slots_sub, 192]
    g_core = blocks * SUBS_PER_BLOCK
    qkv_dev = qkv_dev.reshape(n_cores, g_core * slots_sub, 192)
    lidx_dev = (lidx_flat.reshape(n_cores, g_core, 128, c_sub)
                .transpose(0, 2, 1, 3).reshape(n_cores, 128, g_core * c_sub))
    lidx_dev = np.ascontiguousarray(lidx_dev)
    iota_np = np.tile(np.arange(SUB_NODES, dtype=np.float32), (128, 1))

    meta = dict(bin_of_node=bin_of_node, slot_of_node=slot_of_node, deg=deg,
                c_sub=c_sub, blocks=blocks, n_cores=n_cores)
    in_maps = [
        {"qkv": qkv_dev[c], "lidx": lidx_dev[c], "iota": iota_np}
        for c in range(n_cores)
    ]
    return in_maps, meta


def unshard(results, meta):
    """Gather per-core outputs back to [N_NODES, DK] in original node order."""
    n_cores = meta["n_cores"]
    blocks = meta["blocks"]
    g_core = blocks * SUBS_PER_BLOCK
    allout = np.stack([np.asarray(results[c]["out"]) for c in range(n_cores)])

    bin_of_node = meta["bin_of_node"]
    slot_of_node = meta["slot_of_node"]
    core = bin_of_node // g_core
    g = bin_of_node % g_core
    row = (g // SUBS_PER_BLOCK) * 128 + (g % SUBS_PER_BLOCK) * 32 + slot_of_node
    out_full = allout[core, row].astype(np.float32)
    out_full[meta["deg"] == 0] = 0.0
    return out_full


_program_cache = {}


def kernel(key, value, query, edge_weight_cutoff, edge_index):
    in_maps, meta = prepare(key, value, query, edge_weight_cutoff, edge_index)
    cache_key = (meta["c_sub"], meta["blocks"], meta["n_cores"])
    if cache_key not in _program_cache:
        _program_cache[cache_key] = build_program(*cache_key)
    nc = _program_cache[cache_key]
    res = run_bass_kernel_spmd(nc, in_maps, list(range(meta["n_cores"])))
    return unshard(res.results, meta)
